# revision 41
# baseline (speedup 1.0000x reference)
"""Differential multi-head attention on 8 Trainium2 NeuronCores.

Sharding: tensor-parallel over the 16 heads of both attention branches.
Each core owns 2 heads of branch 1 and the same 2 heads of branch 2
(4 head-units total). The final projection Wp is folded into each
head's output projection on the host (Wo_h @ Wp, with -lambda absorbed
for branch 2), so every core produces a full-shape partial output and
the host unshard step is a plain 8-way sum plus the bias term.

On-chip layout per core (all matmul inputs bf16, fp32 PSUM accum):
  xT   [1024, 4096]   x transposed (d_model on partitions)
  qkT  [128, 4, 2048] per batch: q/k projected; rows 0-63 head A,
                      rows 64-127 head B (d_head on partitions)
  v    [128, 16, 4, 65] per batch: v natural layout per 128-token
                      k-block; last column 1.0 so softmax denominators
                      ride along in the AV matmul output row 64
  scoresT [k,q] via matmul (row-packed K=64 pairs), exp on ScalarE
  (1/sqrt(64) folded into the activation scale), AV accumulates
  o^T[65, q] over 16 k-tiles, denominators inverted with the fast
  approx reciprocal and broadcast across partitions with a rank-1
  fp32r matmul, normalized o^T feeds the fused output projection.
"""

import numpy as np
import ml_dtypes

BF16 = ml_dtypes.bfloat16
H, DH, DIM = 16, 64, 1024
B, S = 2, 2048
N = B * S  # 4096 tokens
SCALE = 1.0 / np.sqrt(DH)
NCORES = 8

_BUILT = {}


def _build_nc():
    import concourse.tile as tile
    from concourse import mybir, bacc

    f32 = mybir.dt.float32
    f32r = mybir.dt.float32r
    bf16 = mybir.dt.bfloat16
    Exp = mybir.ActivationFunctionType.Exp

    nc = bacc.Bacc(None, target_bir_lowering=False, debug=False)

    xT_d = nc.declare_dram_parameter("xT", [DIM, N], bf16, isOutput=False)
    wqk_d = nc.declare_dram_parameter("wqk", [4, DIM, 128], bf16, isOutput=False)
    wv_d = nc.declare_dram_parameter("wv", [DIM, 256], bf16, isOutput=False)
    wf_d = nc.declare_dram_parameter("wf", [2, 128, DIM], bf16, isOutput=False)
    bqk_d = nc.declare_dram_parameter("bqk", [4, 128], f32, isOutput=False)
    bv_d = nc.declare_dram_parameter("bv", [256], f32, isOutput=False)
    out_d = nc.declare_dram_parameter("out", [N, DIM], f32, isOutput=True)

    import concourse.bass as bass
    from contextlib import ExitStack

    with (
        nc.allow_low_precision(reason="bf16 intermediates; PSUM accumulation is fp32"),
        tile.TileContext(nc) as tc,
        ExitStack() as ctx,
    ):
        persist = ctx.enter_context(tc.tile_pool(name="persist", bufs=1))
        exp_pool = ctx.enter_context(tc.tile_pool(name="exp", bufs=3))
        onorm = ctx.enter_context(tc.tile_pool(name="onorm", bufs=4))
        ostage = ctx.enter_context(tc.tile_pool(name="ostage", bufs=3))
        # PSUM pools created AFTER the dedicated qkv(b0) pool is released
        # (see below) so the bank budget is sequenced: 6 banks for the dense
        # opening qkv phase, then 4(scores)+3(av)+1(mm) = 8 for the rest.
        ps_pools = {}

        # ---- persistent SBUF tensors ----
        xT_sb = persist.tile([128, 8, N], bf16, tag="xT")
        wqk_sb = persist.tile([128, 4, 8, 128], bf16, tag="wqk")
        wv_sb = persist.tile([128, 8, 256], bf16, tag="wv")
        wf_sb = persist.tile([128, 2, DIM], bf16, tag="wf")
        bqk_sb = persist.tile([128, 4], f32, tag="bqk")
        bv_sb = persist.tile([128, 256], f32, tag="bv")
        ones_sb = persist.tile([1, 64], bf16, tag="ones")
        qkT_sb = [
            persist.tile([128, 4, S], bf16, name=f"qkT{b}", tag=f"qkT{b}")
            for b in range(B)
        ]
        v_sb = [
            persist.tile([128, 16, 4, 65], bf16, name=f"v{b}", tag=f"v{b}")
            for b in range(B)
        ]
        oT_sb = [
            persist.tile([128, 2, S], bf16, name=f"oT{b}", tag=f"oT{b}")
            for b in range(B)
        ]

        # ---- input DMAs: first matmul group needs wqk + the first token
        # quarter of every xT chunk, so issue those first, split across the
        # two HWDGE issuing engines; the rest streams behind compute.
        dma_engines = [nc.sync, nc.scalar]
        for t in range(4):
            dma_engines[t % 2].dma_start(
                out=wqk_sb[:, t], in_=wqk_d[t].rearrange("(c p) m -> p c m", p=128)
            )
        nc.sync.dma_start(out=bqk_sb, in_=bqk_d.rearrange("t p -> p t"))
        for p in range(4):
            for c in range(8):
                dma_engines[c % 2].dma_start(
                    out=xT_sb[:, c, p * 1024 : (p + 1) * 1024],
                    in_=xT_d[c * 128 : (c + 1) * 128, p * 1024 : (p + 1) * 1024],
                )
            if p == 0:
                nc.scalar.dma_start(
                    out=wv_sb, in_=wv_d.rearrange("(c p) m -> p c m", p=128)
                )
            elif p == 1:
                nc.sync.dma_start(out=wf_sb, in_=wf_d.rearrange("u k n -> k u n"))
                bv_ap = bv_d[:]
                nc.scalar.dma_start(
                    out=bv_sb,
                    in_=bass.AP(
                        tensor=bv_ap.tensor,
                        offset=bv_ap.offset,
                        ap=[[0, 128], *bv_ap.ap],
                    ),
                )

        nc.vector.memset(ones_sb, 1.0)
        for b in range(B):
            nc.vector.memset(v_sb[b][:, :, :, 64:65], 1.0)

        def qk_part(b, nb, pool, tag):
            """q/k projections for one 512-token block of batch b."""
            tok = slice(nb * 512, (nb + 1) * 512)
            gtok = slice(b * S + nb * 512, b * S + (nb + 1) * 512)
            for t in range(4):
                ps = pool.tile([128, 512], f32, name="psqk", tag=tag)
                for c in range(8):
                    nc.tensor.matmul(
                        ps,
                        wqk_sb[:, t, c, :],
                        xT_sb[:, c, gtok],
                        start=(c == 0),
                        stop=(c == 7),
                    )
                nc.vector.tensor_scalar_add(
                    qkT_sb[b][:, t, tok], ps, bqk_sb[:, t : t + 1]
                )

        def v_part(b, part, pool, tag):
            """v projection for four 128-token tiles of batch b."""
            for lb in range(part * 4, part * 4 + 4):
                gtok = slice(b * S + lb * 128, b * S + (lb + 1) * 128)
                ps = pool.tile([128, 256], f32, name="psv", tag=tag)
                for c in range(8):
                    nc.tensor.matmul(
                        ps,
                        xT_sb[:, c, gtok],
                        wv_sb[:, c, :],
                        start=(c == 0),
                        stop=(c == 7),
                    )
                nc.vector.tensor_add(
                    v_sb[b][:, lb, :, 0:64],
                    ps.rearrange("p (u d) -> p u d", u=4),
                    bv_sb.rearrange("p (u d) -> p u d", u=4),
                )

        def qkv_part(b, part, pool, tag):
            qk_part(b, part, pool, tag)
            v_part(b, part, pool, tag)

        def attn_qb(b, qb, weave=None):
            """Both branches of one 512-query block, then its outproj.
            `weave(i)` is invoked after each branch so low-priority filler
            work can be emitted between the ScalarE-critical groups."""
            ps_scores, ps_av, ps_mm = (
                ps_pools["sc"],
                ps_pools["av"],
                ps_pools["mm"],
            )
            qsl = slice(qb * 512, (qb + 1) * 512)
            for br in range(2):
                tq, tk = 2 * br, 2 * br + 1
                po = [
                    ps_av.tile([65, 512], f32, name="poA", tag="av"),
                    ps_av.tile([65, 512], f32, name="poB", tag="av"),
                ]
                for kt in range(16):
                    ksl = slice(kt * 128, (kt + 1) * 128)
                    ps = ps_scores.tile([128, 1024], f32, name="pssc", tag="sc")
                    nc.tensor.matmul(
                        ps[:, 0:512],
                        qkT_sb[b][0:64, tk, ksl],
                        qkT_sb[b][0:64, tq, qsl],
                        start=True,
                        stop=True,
                    )
                    nc.tensor.matmul(
                        ps[:, 512:1024],
                        qkT_sb[b][64:128, tk, ksl],
                        qkT_sb[b][64:128, tq, qsl],
                        start=True,
                        stop=True,
                    )
                    ex = exp_pool.tile([128, 1024], bf16, name="ex", tag="ex")
                    nc.scalar.activation(ex, ps, Exp, scale=float(SCALE))
                    for j in range(2):
                        nc.tensor.matmul(
                            po[j],
                            v_sb[b][:, kt, 2 * br + j, :],
                            ex[:, j * 512 : (j + 1) * 512],
                            start=(kt == 0),
                            stop=(kt == 15),
                        )
                for j in range(2):
                    u = 2 * br + j
                    # copy to SBUF first: releases the AV PSUM slot quickly
                    osb = onorm.tile([65, 512], f32, name="osb", tag="osb")
                    nc.vector.tensor_copy(osb, po[j])
                    # approx reciprocal over the whole tile (the custom DVE op
                    # mis-executes on partition-offset slices); only row 64
                    # (the denominators) is consumed, the rest is discarded.
                    rcp = onorm.tile([65, 512], f32, name="rcp", tag="rcp")
                    nc.vector.reciprocal_approx_fast(out=rcp, in_=osb)
                    rcpb = onorm.tile([1, 512], bf16, name="rcpb", tag="rcpb")
                    nc.vector.tensor_copy(rcpb, rcp[64:65, :])
                    bc_ps = ps_mm.tile([64, 512], f32, name="bcps", tag="mm")
                    nc.tensor.matmul(bc_ps, ones_sb, rcpb, start=True, stop=True)
                    if j == 0:
                        nc.vector.tensor_mul(
                            oT_sb[b][0:64, br, qsl], osb[0:64, :], bc_ps
                        )
                    else:
                        # B-unit: normalize into a temp, then DMA partition-
                        # shift onto rows 64-127 so the output projection can
                        # contract both heads in one K=128 matmul.
                        otmp = onorm.tile([64, 512], bf16, name="otmp", tag="otmp")
                        nc.vector.tensor_mul(otmp, osb[0:64, :], bc_ps)
                        nc.sync.dma_start(
                            out=oT_sb[b][64:128, br, qsl], in_=otmp
                        )
                if weave is not None:
                    weave(br)
            # fused output projection for this query block's 4 token-tiles
            for k4 in range(4):
                lb = qb * 4 + k4
                tsl = slice(lb * 128, (lb + 1) * 128)
                gtok = slice(b * S + lb * 128, b * S + (lb + 1) * 128)
                for nt in range(2):
                    ps = ps_mm.tile([128, 512], f32, name="psop", tag="mm")
                    for ch in range(2):
                        nc.tensor.matmul(
                            ps,
                            oT_sb[b][:, ch, tsl],
                            wf_sb[:, ch, nt * 512 : (nt + 1) * 512],
                            start=(ch == 0),
                            stop=(ch == 1),
                        )
                    ob = ostage.tile([128, 512], f32, name="ob", tag="ob")
                    nc.vector.tensor_copy(ob, ps)
                    nc.sync.dma_start(
                        out=out_d[gtok, nt * 512 : (nt + 1) * 512], in_=ob
                    )

        # batch-0 qkv runs alone with a wide dedicated PSUM pool (dense PE
        # stream, no evacuation stalls); released before the steady pools.
        with tc.tile_pool(name="psq0", bufs=6, space="PSUM") as ps_q0:
            for part in range(4):
                qkv_part(0, part, ps_q0, "q0")
        ps_pools["sc"] = ctx.enter_context(
            tc.tile_pool(name="pss", bufs=2, space="PSUM")
        )
        ps_pools["av"] = ctx.enter_context(
            tc.tile_pool(name="psav", bufs=3, space="PSUM")
        )
        ps_pools["mm"] = ctx.enter_context(
            tc.tile_pool(name="psmm", bufs=1, space="PSUM")
        )
        # batch-1 qkv is woven between batch-0 attention blocks at strictly
        # lower scheduler priority, so it only fills TensorE gaps and never
        # preempts the ScalarE-critical attention chain.
        for qb in range(4):

            def _weave(br, qb=qb):
                if br == 0:
                    qk_part(1, qb, ps_pools["mm"], "mm")
                else:
                    v_part(1, qb, ps_pools["mm"], "mm")

            attn_qb(0, qb, weave=_weave)
        for qb in range(4):
            attn_qb(1, qb)

    nc.compile()
    return nc


def _get_nc():
    if "nc" not in _BUILT:
        _BUILT["nc"] = _build_nc()
    return _BUILT["nc"]


_LAST_RES = None
_RUN_KW = {}  # test harness may set {"trace": True}


def _run_spmd(nc, in_maps):
    global _LAST_RES
    from concourse.bass_utils import run_bass_kernel_spmd

    _LAST_RES = run_bass_kernel_spmd(nc, in_maps, list(range(NCORES)), **_RUN_KW)
    return _LAST_RES


def _get_lambda(lambda_param, layer_idx):
    lf = float(layer_idx)
    exp_term = np.exp(-0.3 * max(lf - 1.0, 0.0))
    init = 0.8 - 0.6 * exp_term
    return float(np.clip(init * float(lambda_param), 0.1, 0.9))


def kernel(**inputs):
    f32 = np.float32
    x = np.asarray(inputs["x"], f32)
    lam = _get_lambda(np.asarray(inputs["lambda_param"], f32).reshape(-1)[0],
                      int(np.asarray(inputs["layer_idx"])))

    Wq = [np.asarray(inputs["Wq1"], f32), np.asarray(inputs["Wq2"], f32)]
    Wk = [np.asarray(inputs["Wk1"], f32), np.asarray(inputs["Wk2"], f32)]
    Wv = [np.asarray(inputs["Wv1"], f32), np.asarray(inputs["Wv2"], f32)]
    Wo = [np.asarray(inputs["Wo1"], f32), np.asarray(inputs["Wo2"], f32)]
    bq = [np.asarray(inputs["bq1"], f32), np.asarray(inputs["bq2"], f32)]
    bk = [np.asarray(inputs["bk1"], f32), np.asarray(inputs["bk2"], f32)]
    bv = [np.asarray(inputs["bv1"], f32), np.asarray(inputs["bv2"], f32)]
    bo = [np.asarray(inputs["bo1"], f32), np.asarray(inputs["bo2"], f32)]
    Wp = np.asarray(inputs["Wp"], f32)
    bp = np.asarray(inputs["bp"], f32)

    xT = np.ascontiguousarray(x.reshape(N, DIM).T).astype(BF16)

    # fused per-head output projections: Wo_h @ Wp (branch 2 absorbs -lambda)
    wf_all = np.einsum("hdo,op->hdp", Wo[0], Wp).astype(f32)
    wf2_all = (-lam) * np.einsum("hdo,op->hdp", Wo[1], Wp).astype(f32)

    in_maps = []
    for c in range(NCORES):
        hs = [2 * c, 2 * c + 1]
        wqk = np.stack(
            [
                Wq[0][:, hs].reshape(DIM, 128),
                Wk[0][:, hs].reshape(DIM, 128),
                Wq[1][:, hs].reshape(DIM, 128),
                Wk[1][:, hs].reshape(DIM, 128),
            ]
        ).astype(BF16)
        wv = np.concatenate(
            [Wv[0][:, hs[0]], Wv[0][:, hs[1]], Wv[1][:, hs[0]], Wv[1][:, hs[1]]],
            axis=1,
        ).astype(BF16)
        wf = np.stack(
            [
                np.concatenate([wf_all[hs[0]], wf_all[hs[1]]], axis=0),
                np.concatenate([wf2_all[hs[0]], wf2_all[hs[1]]], axis=0),
            ]
        ).astype(BF16)
        bqk = np.stack(
            [
                bq[0][hs].reshape(128),
                bk[0][hs].reshape(128),
                bq[1][hs].reshape(128),
                bk[1][hs].reshape(128),
            ]
        ).astype(f32)
        bvv = np.concatenate(
            [bv[0][hs[0]], bv[0][hs[1]], bv[1][hs[0]], bv[1][hs[1]]]
        ).astype(f32)
        in_maps.append(
            {"xT": xT, "wqk": wqk, "wv": wv, "wf": wf, "bqk": bqk, "bv": bvv}
        )

    nc = _get_nc()
    res = _run_spmd(nc, in_maps)

    total = np.zeros((N, DIM), f32)
    for c in range(NCORES):
        total += np.asarray(res.results[c]["out"], f32)
    bias_total = (bo[0] - lam * bo[1]) @ Wp + bp
    total += bias_total[None, :]
    return total.reshape(B, S, DIM).astype(f32)


# revision 43
# speedup vs baseline: 1.0044x; 1.0044x over previous
"""Differential multi-head attention on 8 Trainium2 NeuronCores.

Sharding: tensor-parallel over the 16 heads of both attention branches.
Each core owns 2 heads of branch 1 and the same 2 heads of branch 2
(4 head-units total). The final projection Wp is folded into each
head's output projection on the host (Wo_h @ Wp, with -lambda absorbed
for branch 2), so every core produces a full-shape partial output and
the host unshard step is a plain 8-way sum plus the bias term.

On-chip layout per core (all matmul inputs bf16, fp32 PSUM accum):
  xT   [1024, 4096]   x transposed (d_model on partitions)
  qkT  [128, 4, 2048] per batch: q/k projected; rows 0-63 head A,
                      rows 64-127 head B (d_head on partitions)
  v    [128, 16, 4, 65] per batch: v natural layout per 128-token
                      k-block; last column 1.0 so softmax denominators
                      ride along in the AV matmul output row 64
  scoresT [k,q] via matmul (row-packed K=64 pairs), exp on ScalarE
  (1/sqrt(64) folded into the activation scale), AV accumulates
  o^T[65, q] over 16 k-tiles, denominators inverted with the fast
  approx reciprocal and broadcast across partitions with a rank-1
  fp32r matmul, normalized o^T feeds the fused output projection.
"""

import numpy as np
import ml_dtypes

BF16 = ml_dtypes.bfloat16
H, DH, DIM = 16, 64, 1024
B, S = 2, 2048
N = B * S  # 4096 tokens
SCALE = 1.0 / np.sqrt(DH)
NCORES = 8

_BUILT = {}


def _build_nc():
    import concourse.tile as tile
    from concourse import mybir, bacc

    f32 = mybir.dt.float32
    f32r = mybir.dt.float32r
    bf16 = mybir.dt.bfloat16
    Exp = mybir.ActivationFunctionType.Exp

    nc = bacc.Bacc(None, target_bir_lowering=False, debug=False)

    xT_d = nc.declare_dram_parameter("xT", [DIM, N], bf16, isOutput=False)
    wqk_d = nc.declare_dram_parameter("wqk", [4, DIM, 128], bf16, isOutput=False)
    wv_d = nc.declare_dram_parameter("wv", [DIM, 256], bf16, isOutput=False)
    wf_d = nc.declare_dram_parameter("wf", [2, 128, DIM], bf16, isOutput=False)
    bqk_d = nc.declare_dram_parameter("bqk", [4, 128], f32, isOutput=False)
    bv_d = nc.declare_dram_parameter("bv", [256], f32, isOutput=False)
    out_d = nc.declare_dram_parameter("out", [N, DIM], f32, isOutput=True)

    import concourse.bass as bass
    from contextlib import ExitStack

    with (
        nc.allow_low_precision(reason="bf16 intermediates; PSUM accumulation is fp32"),
        tile.TileContext(nc) as tc,
        ExitStack() as ctx,
    ):
        persist = ctx.enter_context(tc.tile_pool(name="persist", bufs=1))
        exp_pool = ctx.enter_context(tc.tile_pool(name="exp", bufs=4))
        onorm = ctx.enter_context(tc.tile_pool(name="onorm", bufs=4))
        ostage = ctx.enter_context(tc.tile_pool(name="ostage", bufs=3))
        # PSUM pools created AFTER the dedicated qkv(b0) pool is released
        # (see below) so the bank budget is sequenced: 6 banks for the dense
        # opening qkv phase, then 4(scores)+3(av)+1(mm) = 8 for the rest.
        ps_pools = {}

        # ---- persistent SBUF tensors ----
        xT_sb = persist.tile([128, 8, N], bf16, tag="xT")
        wqk_sb = persist.tile([128, 4, 8, 128], bf16, tag="wqk")
        wv_sb = persist.tile([128, 8, 256], bf16, tag="wv")
        wf_sb = persist.tile([128, 2, DIM], bf16, tag="wf")
        bqk_sb = persist.tile([128, 4], f32, tag="bqk")
        bv_sb = persist.tile([128, 256], f32, tag="bv")
        ones_sb = persist.tile([1, 64], bf16, tag="ones")
        qkT_sb = [
            persist.tile([128, 4, S], bf16, name=f"qkT{b}", tag=f"qkT{b}")
            for b in range(B)
        ]
        v_sb = [
            persist.tile([128, 16, 4, 65], bf16, name=f"v{b}", tag=f"v{b}")
            for b in range(B)
        ]
        oT_sb = [
            persist.tile([128, 2, S], bf16, name=f"oT{b}", tag=f"oT{b}")
            for b in range(B)
        ]

        # ---- input DMAs: first matmul group needs wqk + the first token
        # quarter of every xT chunk, so issue those first, split across the
        # two HWDGE issuing engines; the rest streams behind compute.
        dma_engines = [nc.sync, nc.scalar]
        for t in range(4):
            dma_engines[t % 2].dma_start(
                out=wqk_sb[:, t], in_=wqk_d[t].rearrange("(c p) m -> p c m", p=128)
            )
        nc.sync.dma_start(out=bqk_sb, in_=bqk_d.rearrange("t p -> p t"))
        for p in range(4):
            for c in range(8):
                dma_engines[c % 2].dma_start(
                    out=xT_sb[:, c, p * 1024 : (p + 1) * 1024],
                    in_=xT_d[c * 128 : (c + 1) * 128, p * 1024 : (p + 1) * 1024],
                )
            if p == 0:
                nc.scalar.dma_start(
                    out=wv_sb, in_=wv_d.rearrange("(c p) m -> p c m", p=128)
                )
            elif p == 1:
                nc.sync.dma_start(out=wf_sb, in_=wf_d.rearrange("u k n -> k u n"))
                bv_ap = bv_d[:]
                nc.scalar.dma_start(
                    out=bv_sb,
                    in_=bass.AP(
                        tensor=bv_ap.tensor,
                        offset=bv_ap.offset,
                        ap=[[0, 128], *bv_ap.ap],
                    ),
                )

        nc.vector.memset(ones_sb, 1.0)
        for b in range(B):
            nc.vector.memset(v_sb[b][:, :, :, 64:65], 1.0)

        def qk_part(b, nb, pool, tag):
            """q/k projections for one 512-token block of batch b."""
            tok = slice(nb * 512, (nb + 1) * 512)
            gtok = slice(b * S + nb * 512, b * S + (nb + 1) * 512)
            for t in range(4):
                ps = pool.tile([128, 512], f32, name="psqk", tag=tag)
                for c in range(8):
                    nc.tensor.matmul(
                        ps,
                        wqk_sb[:, t, c, :],
                        xT_sb[:, c, gtok],
                        start=(c == 0),
                        stop=(c == 7),
                    )
                nc.vector.tensor_scalar_add(
                    qkT_sb[b][:, t, tok], ps, bqk_sb[:, t : t + 1]
                )

        def v_part(b, part, pool, tag):
            """v projection for four 128-token tiles of batch b."""
            for lb in range(part * 4, part * 4 + 4):
                gtok = slice(b * S + lb * 128, b * S + (lb + 1) * 128)
                ps = pool.tile([128, 256], f32, name="psv", tag=tag)
                for c in range(8):
                    nc.tensor.matmul(
                        ps,
                        xT_sb[:, c, gtok],
                        wv_sb[:, c, :],
                        start=(c == 0),
                        stop=(c == 7),
                    )
                nc.vector.tensor_add(
                    v_sb[b][:, lb, :, 0:64],
                    ps.rearrange("p (u d) -> p u d", u=4),
                    bv_sb.rearrange("p (u d) -> p u d", u=4),
                )

        def qkv_part(b, part, pool, tag):
            qk_part(b, part, pool, tag)
            v_part(b, part, pool, tag)

        def attn_qb(b, qb, weave=None):
            """Both branches of one 512-query block, then its outproj.
            `weave(i)` is invoked after each branch so low-priority filler
            work can be emitted between the ScalarE-critical groups."""
            ps_scores, ps_av, ps_mm = (
                ps_pools["sc"],
                ps_pools["av"],
                ps_pools["mm"],
            )
            qsl = slice(qb * 512, (qb + 1) * 512)
            for br in range(2):
                tq, tk = 2 * br, 2 * br + 1
                po = [
                    ps_av.tile([65, 512], f32, name="poA", tag="av"),
                    ps_av.tile([65, 512], f32, name="poB", tag="av"),
                ]
                for kt in range(16):
                    ksl = slice(kt * 128, (kt + 1) * 128)
                    ps = ps_scores.tile([128, 1024], f32, name="pssc", tag="sc")
                    nc.tensor.matmul(
                        ps[:, 0:512],
                        qkT_sb[b][0:64, tk, ksl],
                        qkT_sb[b][0:64, tq, qsl],
                        start=True,
                        stop=True,
                    )
                    nc.tensor.matmul(
                        ps[:, 512:1024],
                        qkT_sb[b][64:128, tk, ksl],
                        qkT_sb[b][64:128, tq, qsl],
                        start=True,
                        stop=True,
                    )
                    ex = exp_pool.tile([128, 1024], bf16, name="ex", tag="ex")
                    nc.scalar.activation(ex, ps, Exp, scale=float(SCALE))
                    for j in range(2):
                        nc.tensor.matmul(
                            po[j],
                            v_sb[b][:, kt, 2 * br + j, :],
                            ex[:, j * 512 : (j + 1) * 512],
                            start=(kt == 0),
                            stop=(kt == 15),
                        )
                for j in range(2):
                    u = 2 * br + j
                    # copy to SBUF first: releases the AV PSUM slot quickly
                    osb = onorm.tile([65, 512], f32, name="osb", tag="osb")
                    nc.vector.tensor_copy(osb, po[j])
                    # approx reciprocal over the whole tile (the custom DVE op
                    # mis-executes on partition-offset slices); only row 64
                    # (the denominators) is consumed, the rest is discarded.
                    rcp = onorm.tile([65, 512], f32, name="rcp", tag="rcp")
                    nc.vector.reciprocal_approx_fast(out=rcp, in_=osb)
                    rcpb = onorm.tile([1, 512], bf16, name="rcpb", tag="rcpb")
                    nc.vector.tensor_copy(rcpb, rcp[64:65, :])
                    bc_ps = ps_mm.tile([64, 512], f32, name="bcps", tag="mm")
                    nc.tensor.matmul(bc_ps, ones_sb, rcpb, start=True, stop=True)
                    if j == 0:
                        nc.vector.tensor_mul(
                            oT_sb[b][0:64, br, qsl], osb[0:64, :], bc_ps
                        )
                    else:
                        # B-unit: normalize into a temp, then DMA partition-
                        # shift onto rows 64-127 so the output projection can
                        # contract both heads in one K=128 matmul.
                        otmp = onorm.tile([64, 512], bf16, name="otmp", tag="otmp")
                        nc.vector.tensor_mul(otmp, osb[0:64, :], bc_ps)
                        nc.sync.dma_start(
                            out=oT_sb[b][64:128, br, qsl], in_=otmp
                        )
                if weave is not None:
                    weave(br)
            # fused output projection for this query block's 4 token-tiles
            for k4 in range(4):
                lb = qb * 4 + k4
                tsl = slice(lb * 128, (lb + 1) * 128)
                gtok = slice(b * S + lb * 128, b * S + (lb + 1) * 128)
                for nt in range(2):
                    ps = ps_mm.tile([128, 512], f32, name="psop", tag="mm")
                    for ch in range(2):
                        nc.tensor.matmul(
                            ps,
                            oT_sb[b][:, ch, tsl],
                            wf_sb[:, ch, nt * 512 : (nt + 1) * 512],
                            start=(ch == 0),
                            stop=(ch == 1),
                        )
                    ob = ostage.tile([128, 512], f32, name="ob", tag="ob")
                    nc.vector.tensor_copy(ob, ps)
                    nc.sync.dma_start(
                        out=out_d[gtok, nt * 512 : (nt + 1) * 512], in_=ob
                    )

        # batch-0 qkv runs alone with a wide dedicated PSUM pool (dense PE
        # stream, no evacuation stalls); released before the steady pools.
        with tc.tile_pool(name="psq0", bufs=6, space="PSUM") as ps_q0:
            for part in range(4):
                qkv_part(0, part, ps_q0, "q0")
        ps_pools["sc"] = ctx.enter_context(
            tc.tile_pool(name="pss", bufs=2, space="PSUM")
        )
        ps_pools["av"] = ctx.enter_context(
            tc.tile_pool(name="psav", bufs=3, space="PSUM")
        )
        ps_pools["mm"] = ctx.enter_context(
            tc.tile_pool(name="psmm", bufs=1, space="PSUM")
        )
        # batch-1 qkv is woven between batch-0 attention blocks at strictly
        # lower scheduler priority, so it only fills TensorE gaps and never
        # preempts the ScalarE-critical attention chain.
        for qb in range(4):
            attn_qb(0, qb)
            qkv_part(1, qb, ps_pools["mm"], "mm")
        for qb in range(4):
            attn_qb(1, qb)

    nc.compile()
    return nc


def _get_nc():
    if "nc" not in _BUILT:
        _BUILT["nc"] = _build_nc()
    return _BUILT["nc"]


_LAST_RES = None
_RUN_KW = {}  # test harness may set {"trace": True}


def _run_spmd(nc, in_maps):
    global _LAST_RES
    from concourse.bass_utils import run_bass_kernel_spmd

    _LAST_RES = run_bass_kernel_spmd(nc, in_maps, list(range(NCORES)), **_RUN_KW)
    return _LAST_RES


def _get_lambda(lambda_param, layer_idx):
    lf = float(layer_idx)
    exp_term = np.exp(-0.3 * max(lf - 1.0, 0.0))
    init = 0.8 - 0.6 * exp_term
    return float(np.clip(init * float(lambda_param), 0.1, 0.9))


def kernel(**inputs):
    f32 = np.float32
    x = np.asarray(inputs["x"], f32)
    lam = _get_lambda(np.asarray(inputs["lambda_param"], f32).reshape(-1)[0],
                      int(np.asarray(inputs["layer_idx"])))

    Wq = [np.asarray(inputs["Wq1"], f32), np.asarray(inputs["Wq2"], f32)]
    Wk = [np.asarray(inputs["Wk1"], f32), np.asarray(inputs["Wk2"], f32)]
    Wv = [np.asarray(inputs["Wv1"], f32), np.asarray(inputs["Wv2"], f32)]
    Wo = [np.asarray(inputs["Wo1"], f32), np.asarray(inputs["Wo2"], f32)]
    bq = [np.asarray(inputs["bq1"], f32), np.asarray(inputs["bq2"], f32)]
    bk = [np.asarray(inputs["bk1"], f32), np.asarray(inputs["bk2"], f32)]
    bv = [np.asarray(inputs["bv1"], f32), np.asarray(inputs["bv2"], f32)]
    bo = [np.asarray(inputs["bo1"], f32), np.asarray(inputs["bo2"], f32)]
    Wp = np.asarray(inputs["Wp"], f32)
    bp = np.asarray(inputs["bp"], f32)

    xT = np.ascontiguousarray(x.reshape(N, DIM).T).astype(BF16)

    # fused per-head output projections: Wo_h @ Wp (branch 2 absorbs -lambda)
    wf_all = np.einsum("hdo,op->hdp", Wo[0], Wp).astype(f32)
    wf2_all = (-lam) * np.einsum("hdo,op->hdp", Wo[1], Wp).astype(f32)

    in_maps = []
    for c in range(NCORES):
        hs = [2 * c, 2 * c + 1]
        wqk = np.stack(
            [
                Wq[0][:, hs].reshape(DIM, 128),
                Wk[0][:, hs].reshape(DIM, 128),
                Wq[1][:, hs].reshape(DIM, 128),
                Wk[1][:, hs].reshape(DIM, 128),
            ]
        ).astype(BF16)
        wv = np.concatenate(
            [Wv[0][:, hs[0]], Wv[0][:, hs[1]], Wv[1][:, hs[0]], Wv[1][:, hs[1]]],
            axis=1,
        ).astype(BF16)
        wf = np.stack(
            [
                np.concatenate([wf_all[hs[0]], wf_all[hs[1]]], axis=0),
                np.concatenate([wf2_all[hs[0]], wf2_all[hs[1]]], axis=0),
            ]
        ).astype(BF16)
        bqk = np.stack(
            [
                bq[0][hs].reshape(128),
                bk[0][hs].reshape(128),
                bq[1][hs].reshape(128),
                bk[1][hs].reshape(128),
            ]
        ).astype(f32)
        bvv = np.concatenate(
            [bv[0][hs[0]], bv[0][hs[1]], bv[1][hs[0]], bv[1][hs[1]]]
        ).astype(f32)
        in_maps.append(
            {"xT": xT, "wqk": wqk, "wv": wv, "wf": wf, "bqk": bqk, "bv": bvv}
        )

    nc = _get_nc()
    res = _run_spmd(nc, in_maps)

    total = np.zeros((N, DIM), f32)
    for c in range(NCORES):
        total += np.asarray(res.results[c]["out"], f32)
    bias_total = (bo[0] - lam * bo[1]) @ Wp + bp
    total += bias_total[None, :]
    return total.reshape(B, S, DIM).astype(f32)


# revision 44
# speedup vs baseline: 1.0213x; 1.0168x over previous
"""Differential multi-head attention on 8 Trainium2 NeuronCores.

Sharding: tensor-parallel over the 16 heads of both attention branches.
Each core owns 2 heads of branch 1 and the same 2 heads of branch 2
(4 head-units total). The final projection Wp is folded into each
head's output projection on the host (Wo_h @ Wp, with -lambda absorbed
for branch 2), so every core produces a full-shape partial output and
the host unshard step is a plain 8-way sum plus the bias term.

On-chip layout per core (all matmul inputs bf16, fp32 PSUM accum):
  xT   [1024, 4096]   x transposed (d_model on partitions)
  qkT  [128, 4, 2048] per batch: q/k projected; rows 0-63 head A,
                      rows 64-127 head B (d_head on partitions)
  v    [128, 16, 4, 65] per batch: v natural layout per 128-token
                      k-block; last column 1.0 so softmax denominators
                      ride along in the AV matmul output row 64
  scoresT [k,q] via matmul (row-packed K=64 pairs), exp on ScalarE
  (1/sqrt(64) folded into the activation scale), AV accumulates
  o^T[65, q] over 16 k-tiles, denominators inverted with the fast
  approx reciprocal and broadcast across partitions with a rank-1
  fp32r matmul, normalized o^T feeds the fused output projection.
"""

import numpy as np
import ml_dtypes

BF16 = ml_dtypes.bfloat16
H, DH, DIM = 16, 64, 1024
B, S = 2, 2048
N = B * S  # 4096 tokens
SCALE = 1.0 / np.sqrt(DH)
NCORES = 8

_BUILT = {}


def _build_nc():
    import concourse.tile as tile
    from concourse import mybir, bacc

    f32 = mybir.dt.float32
    f32r = mybir.dt.float32r
    bf16 = mybir.dt.bfloat16
    Exp = mybir.ActivationFunctionType.Exp

    nc = bacc.Bacc(None, target_bir_lowering=False, debug=False)

    xT_d = nc.declare_dram_parameter("xT", [DIM, N], bf16, isOutput=False)
    wqk_d = nc.declare_dram_parameter("wqk", [4, DIM, 128], bf16, isOutput=False)
    wv_d = nc.declare_dram_parameter("wv", [DIM, 256], bf16, isOutput=False)
    wf_d = nc.declare_dram_parameter("wf", [2, 128, DIM], bf16, isOutput=False)
    bqk_d = nc.declare_dram_parameter("bqk", [4, 128], f32, isOutput=False)
    bv_d = nc.declare_dram_parameter("bv", [256], f32, isOutput=False)
    out_d = nc.declare_dram_parameter("out", [N, DIM], f32, isOutput=True)

    import concourse.bass as bass
    from contextlib import ExitStack

    with (
        nc.allow_low_precision(reason="bf16 intermediates; PSUM accumulation is fp32"),
        tile.TileContext(nc) as tc,
        ExitStack() as ctx,
    ):
        persist = ctx.enter_context(tc.tile_pool(name="persist", bufs=1))
        exp_pool = ctx.enter_context(tc.tile_pool(name="exp", bufs=3))
        onorm = ctx.enter_context(tc.tile_pool(name="onorm", bufs=4))
        ostage = ctx.enter_context(tc.tile_pool(name="ostage", bufs=3))
        # PSUM pools created AFTER the dedicated qkv(b0) pool is released
        # (see below) so the bank budget is sequenced: 6 banks for the dense
        # opening qkv phase, then 4(scores)+3(av)+1(mm) = 8 for the rest.
        ps_pools = {}

        # ---- persistent SBUF tensors ----
        xT_sb = persist.tile([128, 8, N], bf16, tag="xT")
        wqk_sb = persist.tile([128, 4, 8, 128], bf16, tag="wqk")
        wv_sb = persist.tile([128, 8, 256], bf16, tag="wv")
        wf_sb = persist.tile([128, 2, DIM], bf16, tag="wf")
        bqk_sb = persist.tile([128, 4], f32, tag="bqk")
        bv_sb = persist.tile([128, 256], f32, tag="bv")
        ones_sb = persist.tile([1, 64], bf16, tag="ones")
        qkT_sb = [
            persist.tile([128, 4, S], bf16, name=f"qkT{b}", tag=f"qkT{b}")
            for b in range(B)
        ]
        v_sb = [
            persist.tile([128, 16, 4, 65], bf16, name=f"v{b}", tag=f"v{b}")
            for b in range(B)
        ]
        oT_sb = [
            persist.tile([128, 2, S], bf16, name=f"oT{b}", tag=f"oT{b}")
            for b in range(B)
        ]

        # ---- input DMAs: first matmul group needs wqk + the first token
        # quarter of every xT chunk, so issue those first, split across the
        # two HWDGE issuing engines; the rest streams behind compute.
        dma_engines = [nc.sync, nc.scalar]
        for t in range(4):
            dma_engines[t % 2].dma_start(
                out=wqk_sb[:, t], in_=wqk_d[t].rearrange("(c p) m -> p c m", p=128)
            )
        nc.sync.dma_start(out=bqk_sb, in_=bqk_d.rearrange("t p -> p t"))
        for p in range(4):
            for c in range(8):
                dma_engines[c % 2].dma_start(
                    out=xT_sb[:, c, p * 1024 : (p + 1) * 1024],
                    in_=xT_d[c * 128 : (c + 1) * 128, p * 1024 : (p + 1) * 1024],
                )
            if p == 0:
                nc.scalar.dma_start(
                    out=wv_sb, in_=wv_d.rearrange("(c p) m -> p c m", p=128)
                )
            elif p == 1:
                nc.sync.dma_start(out=wf_sb, in_=wf_d.rearrange("u k n -> k u n"))
                bv_ap = bv_d[:]
                nc.scalar.dma_start(
                    out=bv_sb,
                    in_=bass.AP(
                        tensor=bv_ap.tensor,
                        offset=bv_ap.offset,
                        ap=[[0, 128], *bv_ap.ap],
                    ),
                )

        nc.vector.memset(ones_sb, 1.0)
        for b in range(B):
            nc.vector.memset(v_sb[b][:, :, :, 64:65], 1.0)

        def qk_part(b, nb, pool, tag):
            """q/k projections for one 512-token block of batch b."""
            tok = slice(nb * 512, (nb + 1) * 512)
            gtok = slice(b * S + nb * 512, b * S + (nb + 1) * 512)
            for t in range(4):
                ps = pool.tile([128, 512], f32, name="psqk", tag=tag)
                for c in range(8):
                    nc.tensor.matmul(
                        ps,
                        wqk_sb[:, t, c, :],
                        xT_sb[:, c, gtok],
                        start=(c == 0),
                        stop=(c == 7),
                    )
                nc.vector.tensor_scalar_add(
                    qkT_sb[b][:, t, tok], ps, bqk_sb[:, t : t + 1]
                )

        def v_part(b, part, pool, tag):
            """v projection for four 128-token tiles of batch b."""
            for lb in range(part * 4, part * 4 + 4):
                gtok = slice(b * S + lb * 128, b * S + (lb + 1) * 128)
                ps = pool.tile([128, 256], f32, name="psv", tag=tag)
                for c in range(8):
                    nc.tensor.matmul(
                        ps,
                        xT_sb[:, c, gtok],
                        wv_sb[:, c, :],
                        start=(c == 0),
                        stop=(c == 7),
                    )
                nc.vector.tensor_add(
                    v_sb[b][:, lb, :, 0:64],
                    ps.rearrange("p (u d) -> p u d", u=4),
                    bv_sb.rearrange("p (u d) -> p u d", u=4),
                )

        def qkv_part(b, part, pool, tag):
            qk_part(b, part, pool, tag)
            v_part(b, part, pool, tag)

        def attn_qb(b, qb, weave=None):
            """Both branches of one 512-query block, then its outproj.
            `weave(i)` is invoked after each branch so low-priority filler
            work can be emitted between the ScalarE-critical groups."""
            ps_scores, ps_av, ps_mm = (
                ps_pools["sc"],
                ps_pools["av"],
                ps_pools["mm"],
            )
            qsl = slice(qb * 512, (qb + 1) * 512)
            for br in range(2):
                tq, tk = 2 * br, 2 * br + 1
                po = [
                    ps_av.tile([65, 512], f32, name="poA", tag="av"),
                    ps_av.tile([65, 512], f32, name="poB", tag="av"),
                ]
                for kt in range(16):
                    ksl = slice(kt * 128, (kt + 1) * 128)
                    ps = ps_scores.tile([128, 1024], f32, name="pssc", tag="sc")
                    nc.tensor.matmul(
                        ps[:, 0:512],
                        qkT_sb[b][0:64, tk, ksl],
                        qkT_sb[b][0:64, tq, qsl],
                        start=True,
                        stop=True,
                    )
                    nc.tensor.matmul(
                        ps[:, 512:1024],
                        qkT_sb[b][64:128, tk, ksl],
                        qkT_sb[b][64:128, tq, qsl],
                        start=True,
                        stop=True,
                    )
                    ex = exp_pool.tile([128, 1024], bf16, name="ex", tag="ex")
                    nc.scalar.activation(ex, ps, Exp, scale=float(SCALE))
                    for j in range(2):
                        nc.tensor.matmul(
                            po[j],
                            v_sb[b][:, kt, 2 * br + j, :],
                            ex[:, j * 512 : (j + 1) * 512],
                            start=(kt == 0),
                            stop=(kt == 15),
                        )
                for j in range(2):
                    u = 2 * br + j
                    # copy to SBUF first: releases the AV PSUM slot quickly
                    osb = onorm.tile([65, 512], f32, name="osb", tag="osb")
                    nc.vector.tensor_copy(osb, po[j])
                    # approx reciprocal over the whole tile (the custom DVE op
                    # mis-executes on partition-offset slices); only row 64
                    # (the denominators) is consumed, the rest is discarded.
                    rcp = onorm.tile([65, 512], f32, name="rcp", tag="rcp")
                    nc.vector.reciprocal_approx_fast(out=rcp, in_=osb)
                    rcpb = onorm.tile([1, 512], bf16, name="rcpb", tag="rcpb")
                    nc.vector.tensor_copy(rcpb, rcp[64:65, :])
                    bc_ps = ps_mm.tile([64, 512], f32, name="bcps", tag="mm")
                    nc.tensor.matmul(bc_ps, ones_sb, rcpb, start=True, stop=True)
                    if j == 0:
                        nc.vector.tensor_mul(
                            oT_sb[b][0:64, br, qsl], osb[0:64, :], bc_ps
                        )
                    else:
                        # B-unit: normalize into a temp, then DMA partition-
                        # shift onto rows 64-127 so the output projection can
                        # contract both heads in one K=128 matmul.
                        otmp = onorm.tile([64, 512], bf16, name="otmp", tag="otmp")
                        nc.vector.tensor_mul(otmp, osb[0:64, :], bc_ps)
                        nc.sync.dma_start(
                            out=oT_sb[b][64:128, br, qsl], in_=otmp
                        )
                if weave is not None:
                    weave(br)
            # fused output projection for this query block's 4 token-tiles
            for k4 in range(4):
                lb = qb * 4 + k4
                tsl = slice(lb * 128, (lb + 1) * 128)
                gtok = slice(b * S + lb * 128, b * S + (lb + 1) * 128)
                for nt in range(2):
                    ps = ps_mm.tile([128, 512], f32, name="psop", tag="mm")
                    for ch in range(2):
                        nc.tensor.matmul(
                            ps,
                            oT_sb[b][:, ch, tsl],
                            wf_sb[:, ch, nt * 512 : (nt + 1) * 512],
                            start=(ch == 0),
                            stop=(ch == 1),
                        )
                    ob = ostage.tile([128, 512], f32, name="ob", tag="ob")
                    nc.vector.tensor_copy(ob, ps)
                    nc.sync.dma_start(
                        out=out_d[gtok, nt * 512 : (nt + 1) * 512], in_=ob
                    )

        # batch-0 qkv runs alone with a wide dedicated PSUM pool (dense PE
        # stream, no evacuation stalls); released before the steady pools.
        with tc.tile_pool(name="psq0", bufs=6, space="PSUM") as ps_q0:
            for part in range(4):
                qkv_part(0, part, ps_q0, "q0")
        ps_pools["sc"] = ctx.enter_context(
            tc.tile_pool(name="pss", bufs=2, space="PSUM")
        )
        ps_pools["av"] = ctx.enter_context(
            tc.tile_pool(name="psav", bufs=3, space="PSUM")
        )
        ps_pools["mm"] = ctx.enter_context(
            tc.tile_pool(name="psmm", bufs=1, space="PSUM")
        )
        # batch-1 qkv is woven between batch-0 attention blocks at strictly
        # lower scheduler priority, so it only fills TensorE gaps and never
        # preempts the ScalarE-critical attention chain.
        for qb in range(4):
            attn_qb(0, qb)
            qkv_part(1, qb, ps_pools["mm"], "mm")
        for qb in range(4):
            attn_qb(1, qb)

    nc.compile()
    return nc


def _get_nc():
    if "nc" not in _BUILT:
        _BUILT["nc"] = _build_nc()
    return _BUILT["nc"]


_LAST_RES = None
_RUN_KW = {}  # test harness may set {"trace": True}


def _run_spmd(nc, in_maps):
    global _LAST_RES
    from concourse.bass_utils import run_bass_kernel_spmd

    _LAST_RES = run_bass_kernel_spmd(nc, in_maps, list(range(NCORES)), **_RUN_KW)
    return _LAST_RES


def _get_lambda(lambda_param, layer_idx):
    lf = float(layer_idx)
    exp_term = np.exp(-0.3 * max(lf - 1.0, 0.0))
    init = 0.8 - 0.6 * exp_term
    return float(np.clip(init * float(lambda_param), 0.1, 0.9))


def kernel(**inputs):
    f32 = np.float32
    x = np.asarray(inputs["x"], f32)
    lam = _get_lambda(np.asarray(inputs["lambda_param"], f32).reshape(-1)[0],
                      int(np.asarray(inputs["layer_idx"])))

    Wq = [np.asarray(inputs["Wq1"], f32), np.asarray(inputs["Wq2"], f32)]
    Wk = [np.asarray(inputs["Wk1"], f32), np.asarray(inputs["Wk2"], f32)]
    Wv = [np.asarray(inputs["Wv1"], f32), np.asarray(inputs["Wv2"], f32)]
    Wo = [np.asarray(inputs["Wo1"], f32), np.asarray(inputs["Wo2"], f32)]
    bq = [np.asarray(inputs["bq1"], f32), np.asarray(inputs["bq2"], f32)]
    bk = [np.asarray(inputs["bk1"], f32), np.asarray(inputs["bk2"], f32)]
    bv = [np.asarray(inputs["bv1"], f32), np.asarray(inputs["bv2"], f32)]
    bo = [np.asarray(inputs["bo1"], f32), np.asarray(inputs["bo2"], f32)]
    Wp = np.asarray(inputs["Wp"], f32)
    bp = np.asarray(inputs["bp"], f32)

    xT = np.ascontiguousarray(x.reshape(N, DIM).T).astype(BF16)

    # fused per-head output projections: Wo_h @ Wp (branch 2 absorbs -lambda)
    wf_all = np.einsum("hdo,op->hdp", Wo[0], Wp).astype(f32)
    wf2_all = (-lam) * np.einsum("hdo,op->hdp", Wo[1], Wp).astype(f32)

    in_maps = []
    for c in range(NCORES):
        hs = [2 * c, 2 * c + 1]
        wqk = np.stack(
            [
                Wq[0][:, hs].reshape(DIM, 128),
                Wk[0][:, hs].reshape(DIM, 128),
                Wq[1][:, hs].reshape(DIM, 128),
                Wk[1][:, hs].reshape(DIM, 128),
            ]
        ).astype(BF16)
        wv = np.concatenate(
            [Wv[0][:, hs[0]], Wv[0][:, hs[1]], Wv[1][:, hs[0]], Wv[1][:, hs[1]]],
            axis=1,
        ).astype(BF16)
        wf = np.stack(
            [
                np.concatenate([wf_all[hs[0]], wf_all[hs[1]]], axis=0),
                np.concatenate([wf2_all[hs[0]], wf2_all[hs[1]]], axis=0),
            ]
        ).astype(BF16)
        bqk = np.stack(
            [
                bq[0][hs].reshape(128),
                bk[0][hs].reshape(128),
                bq[1][hs].reshape(128),
                bk[1][hs].reshape(128),
            ]
        ).astype(f32)
        bvv = np.concatenate(
            [bv[0][hs[0]], bv[0][hs[1]], bv[1][hs[0]], bv[1][hs[1]]]
        ).astype(f32)
        in_maps.append(
            {"xT": xT, "wqk": wqk, "wv": wv, "wf": wf, "bqk": bqk, "bv": bvv}
        )

    nc = _get_nc()
    res = _run_spmd(nc, in_maps)

    total = np.zeros((N, DIM), f32)
    for c in range(NCORES):
        total += np.asarray(res.results[c]["out"], f32)
    bias_total = (bo[0] - lam * bo[1]) @ Wp + bp
    total += bias_total[None, :]
    return total.reshape(B, S, DIM).astype(f32)


# revision 45
# speedup vs baseline: 1.0423x; 1.0206x over previous
"""Differential multi-head attention on 8 Trainium2 NeuronCores.

Sharding: tensor-parallel over the 16 heads of both attention branches.
Each core owns 2 heads of branch 1 and the same 2 heads of branch 2
(4 head-units total). The final projection Wp is folded into each
head's output projection on the host (Wo_h @ Wp, with -lambda absorbed
for branch 2), so every core produces a full-shape partial output and
the host unshard step is a plain 8-way sum plus the bias term.

On-chip layout per core (all matmul inputs bf16, fp32 PSUM accum):
  xT   [1024, 4096]   x transposed (d_model on partitions)
  qkT  [128, 4, 2048] per batch: q/k projected; rows 0-63 head A,
                      rows 64-127 head B (d_head on partitions)
  v    [128, 16, 4, 65] per batch: v natural layout per 128-token
                      k-block; last column 1.0 so softmax denominators
                      ride along in the AV matmul output row 64
  scoresT [k,q] via matmul (row-packed K=64 pairs), exp on ScalarE
  (1/sqrt(64) folded into the activation scale), AV accumulates
  o^T[65, q] over 16 k-tiles, denominators inverted with the fast
  approx reciprocal and broadcast across partitions with a rank-1
  fp32r matmul, normalized o^T feeds the fused output projection.
"""

import numpy as np
import ml_dtypes

BF16 = ml_dtypes.bfloat16
H, DH, DIM = 16, 64, 1024
B, S = 2, 2048
N = B * S  # 4096 tokens
SCALE = 1.0 / np.sqrt(DH)
NCORES = 8

_BUILT = {}


def _build_nc():
    import concourse.tile as tile
    from concourse import mybir, bacc

    f32 = mybir.dt.float32
    f32r = mybir.dt.float32r
    bf16 = mybir.dt.bfloat16
    Exp = mybir.ActivationFunctionType.Exp

    nc = bacc.Bacc(None, target_bir_lowering=False, debug=False)

    xT_d = nc.declare_dram_parameter("xT", [DIM, N], bf16, isOutput=False)
    wqk_d = nc.declare_dram_parameter("wqk", [4, DIM, 128], bf16, isOutput=False)
    wv_d = nc.declare_dram_parameter("wv", [DIM, 256], bf16, isOutput=False)
    wf_d = nc.declare_dram_parameter("wf", [2, 128, DIM], bf16, isOutput=False)
    bqk_d = nc.declare_dram_parameter("bqk", [4, 128], f32, isOutput=False)
    bv_d = nc.declare_dram_parameter("bv", [256], f32, isOutput=False)
    out_d = nc.declare_dram_parameter("out", [N, DIM], f32, isOutput=True)

    import concourse.bass as bass
    from contextlib import ExitStack

    with (
        nc.allow_low_precision(reason="bf16 intermediates; PSUM accumulation is fp32"),
        tile.TileContext(nc) as tc,
        ExitStack() as ctx,
    ):
        persist = ctx.enter_context(tc.tile_pool(name="persist", bufs=1))
        exp_pool = ctx.enter_context(tc.tile_pool(name="exp", bufs=3))
        onorm = ctx.enter_context(tc.tile_pool(name="onorm", bufs=4))
        ostage = ctx.enter_context(tc.tile_pool(name="ostage", bufs=3))
        # PSUM pools created AFTER the dedicated qkv(b0) pool is released
        # (see below) so the bank budget is sequenced: 6 banks for the dense
        # opening qkv phase, then 4(scores)+3(av)+1(mm) = 8 for the rest.
        ps_pools = {}

        # ---- persistent SBUF tensors ----
        xT_sb = persist.tile([128, 8, N], bf16, tag="xT")
        wqk_sb = persist.tile([128, 4, 8, 128], bf16, tag="wqk")
        wv_sb = persist.tile([128, 8, 256], bf16, tag="wv")
        wf_sb = persist.tile([128, 2, DIM], bf16, tag="wf")
        bqk_sb = persist.tile([128, 4], f32, tag="bqk")
        bv_sb = persist.tile([128, 256], f32, tag="bv")
        ones_sb = persist.tile([1, 64], bf16, tag="ones")
        qkT_sb = [
            persist.tile([128, 4, S], bf16, name=f"qkT{b}", tag=f"qkT{b}")
            for b in range(B)
        ]
        v_sb = [
            persist.tile([128, 16, 4, 65], bf16, name=f"v{b}", tag=f"v{b}")
            for b in range(B)
        ]
        oT_sb = [
            persist.tile([128, 2, S], bf16, name=f"oT{b}", tag=f"oT{b}")
            for b in range(B)
        ]

        # ---- input DMAs: first matmul group needs wqk + the first token
        # quarter of every xT chunk, so issue those first, split across the
        # two HWDGE issuing engines; the rest streams behind compute.
        dma_engines = [nc.sync, nc.scalar]
        for t in range(4):
            dma_engines[t % 2].dma_start(
                out=wqk_sb[:, t], in_=wqk_d[t].rearrange("(c p) m -> p c m", p=128)
            )
        nc.sync.dma_start(out=bqk_sb, in_=bqk_d.rearrange("t p -> p t"))
        for p in range(4):
            for c in range(8):
                dma_engines[c % 2].dma_start(
                    out=xT_sb[:, c, p * 1024 : (p + 1) * 1024],
                    in_=xT_d[c * 128 : (c + 1) * 128, p * 1024 : (p + 1) * 1024],
                )
            if p == 0:
                nc.scalar.dma_start(
                    out=wv_sb, in_=wv_d.rearrange("(c p) m -> p c m", p=128)
                )
            elif p == 1:
                nc.sync.dma_start(out=wf_sb, in_=wf_d.rearrange("u k n -> k u n"))
                bv_ap = bv_d[:]
                nc.scalar.dma_start(
                    out=bv_sb,
                    in_=bass.AP(
                        tensor=bv_ap.tensor,
                        offset=bv_ap.offset,
                        ap=[[0, 128], *bv_ap.ap],
                    ),
                )

        nc.vector.memset(ones_sb, 1.0)
        for b in range(B):
            nc.vector.memset(v_sb[b][:, :, :, 64:65], 1.0)

        def qk_part(b, nb, pool, tag):
            """q/k projections for one 512-token block of batch b."""
            tok = slice(nb * 512, (nb + 1) * 512)
            gtok = slice(b * S + nb * 512, b * S + (nb + 1) * 512)
            for t in range(4):
                ps = pool.tile([128, 512], f32, name="psqk", tag=tag)
                for c in range(8):
                    nc.tensor.matmul(
                        ps,
                        wqk_sb[:, t, c, :],
                        xT_sb[:, c, gtok],
                        start=(c == 0),
                        stop=(c == 7),
                    )
                nc.vector.tensor_scalar_add(
                    qkT_sb[b][:, t, tok], ps, bqk_sb[:, t : t + 1]
                )

        def v_part(b, part, pool, tag):
            """v projection for four 128-token tiles of batch b."""
            for lb in range(part * 4, part * 4 + 4):
                gtok = slice(b * S + lb * 128, b * S + (lb + 1) * 128)
                ps = pool.tile([128, 256], f32, name="psv", tag=tag)
                for c in range(8):
                    nc.tensor.matmul(
                        ps,
                        xT_sb[:, c, gtok],
                        wv_sb[:, c, :],
                        start=(c == 0),
                        stop=(c == 7),
                    )
                nc.vector.tensor_add(
                    v_sb[b][:, lb, :, 0:64],
                    ps.rearrange("p (u d) -> p u d", u=4),
                    bv_sb.rearrange("p (u d) -> p u d", u=4),
                )

        def qkv_part(b, part, pool, tag):
            qk_part(b, part, pool, tag)
            v_part(b, part, pool, tag)

        def attn_qb(b, qb, weave=None):
            """Both branches of one 512-query block, then its outproj.
            `weave(i)` is invoked after each branch so low-priority filler
            work can be emitted between the ScalarE-critical groups."""
            ps_scores, ps_av, ps_mm = (
                ps_pools["sc"],
                ps_pools["av"],
                ps_pools["mm"],
            )
            qsl = slice(qb * 512, (qb + 1) * 512)
            for br in range(2):
                tq, tk = 2 * br, 2 * br + 1
                po = [
                    ps_av.tile([65, 512], f32, name="poA", tag="av"),
                    ps_av.tile([65, 512], f32, name="poB", tag="av"),
                ]
                for kt in range(16):
                    ksl = slice(kt * 128, (kt + 1) * 128)
                    ps = ps_scores.tile([128, 1024], f32, name="pssc", tag="sc")
                    nc.tensor.matmul(
                        ps[:, 0:512],
                        qkT_sb[b][0:64, tk, ksl],
                        qkT_sb[b][0:64, tq, qsl],
                        start=True,
                        stop=True,
                    )
                    nc.tensor.matmul(
                        ps[:, 512:1024],
                        qkT_sb[b][64:128, tk, ksl],
                        qkT_sb[b][64:128, tq, qsl],
                        start=True,
                        stop=True,
                    )
                    ex = exp_pool.tile([128, 1024], bf16, name="ex", tag="ex")
                    nc.scalar.activation(ex, ps, Exp, scale=float(SCALE))
                    for j in range(2):
                        nc.tensor.matmul(
                            po[j],
                            v_sb[b][:, kt, 2 * br + j, :],
                            ex[:, j * 512 : (j + 1) * 512],
                            start=(kt == 0),
                            stop=(kt == 15),
                        )
                for j in range(2):
                    u = 2 * br + j
                    # copy to SBUF first: releases the AV PSUM slot quickly
                    osb = onorm.tile([65, 512], f32, name="osb", tag="osb")
                    nc.vector.tensor_copy(osb, po[j])
                    # approx reciprocal over the whole tile (the custom DVE op
                    # mis-executes on partition-offset slices); only row 64
                    # (the denominators) is consumed, the rest is discarded.
                    rcp = onorm.tile([65, 512], f32, name="rcp", tag="rcp")
                    nc.vector.reciprocal_approx_fast(out=rcp, in_=osb)
                    rcpb = onorm.tile([1, 512], bf16, name="rcpb", tag="rcpb")
                    nc.vector.tensor_copy(rcpb, rcp[64:65, :])
                    bc_ps = ps_mm.tile([64, 512], f32, name="bcps", tag="mm")
                    nc.tensor.matmul(bc_ps, ones_sb, rcpb, start=True, stop=True)
                    if j == 0:
                        nc.vector.tensor_mul(
                            oT_sb[b][0:64, br, qsl], osb[0:64, :], bc_ps
                        )
                    else:
                        # B-unit: normalize into a temp, then DMA partition-
                        # shift onto rows 64-127 so the output projection can
                        # contract both heads in one K=128 matmul.
                        otmp = onorm.tile([64, 512], bf16, name="otmp", tag="otmp")
                        nc.vector.tensor_mul(otmp, osb[0:64, :], bc_ps)
                        nc.sync.dma_start(
                            out=oT_sb[b][64:128, br, qsl], in_=otmp
                        )
                if weave is not None:
                    weave(br)
            # fused output projection for this query block's 4 token-tiles
            for k4 in range(4):
                lb = qb * 4 + k4
                tsl = slice(lb * 128, (lb + 1) * 128)
                gtok = slice(b * S + lb * 128, b * S + (lb + 1) * 128)
                for nt in range(2):
                    ps = ps_mm.tile([128, 512], f32, name="psop", tag="mm")
                    for ch in range(2):
                        nc.tensor.matmul(
                            ps,
                            oT_sb[b][:, ch, tsl],
                            wf_sb[:, ch, nt * 512 : (nt + 1) * 512],
                            start=(ch == 0),
                            stop=(ch == 1),
                        )
                    ob = ostage.tile([128, 512], f32, name="ob", tag="ob")
                    nc.vector.tensor_copy(ob, ps)
                    nc.sync.dma_start(
                        out=out_d[gtok, nt * 512 : (nt + 1) * 512], in_=ob
                    )

        # batch-0 qkv runs alone with a wide dedicated PSUM pool (dense PE
        # stream, no evacuation stalls); released before the steady pools.
        with tc.tile_pool(name="psq0", bufs=6, space="PSUM") as ps_q0:
            for part in range(4):
                qkv_part(0, part, ps_q0, "q0")
        ps_pools["sc"] = ctx.enter_context(
            tc.tile_pool(name="pss", bufs=2, space="PSUM")
        )
        ps_pools["av"] = ctx.enter_context(
            tc.tile_pool(name="psav", bufs=3, space="PSUM")
        )
        ps_pools["mm"] = ctx.enter_context(
            tc.tile_pool(name="psmm", bufs=1, space="PSUM")
        )
        # batch-1 qkv is woven between batch-0 attention blocks at strictly
        # lower scheduler priority, so it only fills TensorE gaps and never
        # preempts the ScalarE-critical attention chain.
        # front-load batch-1 qkv into the first attention blocks so the last
        # part is never exposed at the batch transition
        for qb in range(4):
            attn_qb(0, qb)
            if qb == 0:
                qkv_part(1, 0, ps_pools["mm"], "mm")
                qkv_part(1, 1, ps_pools["mm"], "mm")
            elif qb == 1:
                qkv_part(1, 2, ps_pools["mm"], "mm")
            elif qb == 2:
                qkv_part(1, 3, ps_pools["mm"], "mm")
        for qb in range(4):
            attn_qb(1, qb)

    nc.compile()
    return nc


def _get_nc():
    if "nc" not in _BUILT:
        _BUILT["nc"] = _build_nc()
    return _BUILT["nc"]


_LAST_RES = None
_RUN_KW = {}  # test harness may set {"trace": True}


def _run_spmd(nc, in_maps):
    global _LAST_RES
    from concourse.bass_utils import run_bass_kernel_spmd

    _LAST_RES = run_bass_kernel_spmd(nc, in_maps, list(range(NCORES)), **_RUN_KW)
    return _LAST_RES


def _get_lambda(lambda_param, layer_idx):
    lf = float(layer_idx)
    exp_term = np.exp(-0.3 * max(lf - 1.0, 0.0))
    init = 0.8 - 0.6 * exp_term
    return float(np.clip(init * float(lambda_param), 0.1, 0.9))


def kernel(**inputs):
    f32 = np.float32
    x = np.asarray(inputs["x"], f32)
    lam = _get_lambda(np.asarray(inputs["lambda_param"], f32).reshape(-1)[0],
                      int(np.asarray(inputs["layer_idx"])))

    Wq = [np.asarray(inputs["Wq1"], f32), np.asarray(inputs["Wq2"], f32)]
    Wk = [np.asarray(inputs["Wk1"], f32), np.asarray(inputs["Wk2"], f32)]
    Wv = [np.asarray(inputs["Wv1"], f32), np.asarray(inputs["Wv2"], f32)]
    Wo = [np.asarray(inputs["Wo1"], f32), np.asarray(inputs["Wo2"], f32)]
    bq = [np.asarray(inputs["bq1"], f32), np.asarray(inputs["bq2"], f32)]
    bk = [np.asarray(inputs["bk1"], f32), np.asarray(inputs["bk2"], f32)]
    bv = [np.asarray(inputs["bv1"], f32), np.asarray(inputs["bv2"], f32)]
    bo = [np.asarray(inputs["bo1"], f32), np.asarray(inputs["bo2"], f32)]
    Wp = np.asarray(inputs["Wp"], f32)
    bp = np.asarray(inputs["bp"], f32)

    xT = np.ascontiguousarray(x.reshape(N, DIM).T).astype(BF16)

    # fused per-head output projections: Wo_h @ Wp (branch 2 absorbs -lambda)
    wf_all = np.einsum("hdo,op->hdp", Wo[0], Wp).astype(f32)
    wf2_all = (-lam) * np.einsum("hdo,op->hdp", Wo[1], Wp).astype(f32)

    in_maps = []
    for c in range(NCORES):
        hs = [2 * c, 2 * c + 1]
        wqk = np.stack(
            [
                Wq[0][:, hs].reshape(DIM, 128),
                Wk[0][:, hs].reshape(DIM, 128),
                Wq[1][:, hs].reshape(DIM, 128),
                Wk[1][:, hs].reshape(DIM, 128),
            ]
        ).astype(BF16)
        wv = np.concatenate(
            [Wv[0][:, hs[0]], Wv[0][:, hs[1]], Wv[1][:, hs[0]], Wv[1][:, hs[1]]],
            axis=1,
        ).astype(BF16)
        wf = np.stack(
            [
                np.concatenate([wf_all[hs[0]], wf_all[hs[1]]], axis=0),
                np.concatenate([wf2_all[hs[0]], wf2_all[hs[1]]], axis=0),
            ]
        ).astype(BF16)
        bqk = np.stack(
            [
                bq[0][hs].reshape(128),
                bk[0][hs].reshape(128),
                bq[1][hs].reshape(128),
                bk[1][hs].reshape(128),
            ]
        ).astype(f32)
        bvv = np.concatenate(
            [bv[0][hs[0]], bv[0][hs[1]], bv[1][hs[0]], bv[1][hs[1]]]
        ).astype(f32)
        in_maps.append(
            {"xT": xT, "wqk": wqk, "wv": wv, "wf": wf, "bqk": bqk, "bv": bvv}
        )

    nc = _get_nc()
    res = _run_spmd(nc, in_maps)

    total = np.zeros((N, DIM), f32)
    for c in range(NCORES):
        total += np.asarray(res.results[c]["out"], f32)
    bias_total = (bo[0] - lam * bo[1]) @ Wp + bp
    total += bias_total[None, :]
    return total.reshape(B, S, DIM).astype(f32)
